# revision 26
# baseline (speedup 1.0000x reference)
"""Locally-connected 2D layer on 8 Trainium2 NeuronCores.

Problem: x[128,3,64,64] f32, per-position weights W[60,60,32,75], bias b[60,60,32]
  out[b,o,y,x] = sum_k patches[b,y,x,k] * W[y,x,o,k] + b[y,x,o],  k=(c,dy,dx)

Strategy (spatial sharding over output rows, 8 rows/core, memory-regime):
  - mod-8 ring of input-row patch planes on SBUF partitions 0..119 (8 slots x
    15 (c,dx)-planes), ones row at partition 120 -> contraction K=121.  Input
    row r lives in slot r%8; output row k multiplies slots k..k+4 (mod 8) with
    nonzero weights and the other 45 plane rows with zeros, so the rhs window
    is always the full fixed [0,121) partition range - no ring rotation, no
    wraparound, no SBUF->SBUF copies.
  - W is stored per-row UNPADDED in HBM ([75,1920] per output row) and DMA'd
    into a zero-memset [121, 8*1920] tile at the row's (possibly wrapped)
    partition stripes; bias is one [1, 8*1920] row at partition 120 (ones).
  - Ring advances (input rows 8..11 -> slots 0..3) are plain HBM loads with
    ~3 output rows of scheduling slack, column-halved and issued right after
    the last reader's matmuls.
  - All DMAs are large-ish and spread over the sync/scalar/gpsimd queues for
    parallel dispatch + deeper SDMA pipelining (~250 GB/s aggregate).
  - Per output row: 15 groups of 4 column-tiled matmuls (lhsT=W[121,32],
    rhs=XP[121,128] -> out[32o,128b] at PSUM partitions 32j); PSUM->SBUF
    copies (f32->bf16) rotate over vector/scalar/gpsimd; bf16 stores per pair.
"""

import numpy as np

B, C, H, WIDTH = 128, 3, 64, 64
KH = KW = 5
RY = RX = 60
O = 32
NCORES = 8
RPC = 8             # output rows computed per core (8*8=64, last 4 dropped)
NSLOT = 8           # ring slots; contraction = 8*15 + 1(ones) = 121
NPL = KW * C        # 15 planes per input row
KC = NSLOT * NPL + 1  # 121 live contraction rows (ones at 120)
KP = 128            # tile partition count; DMAs at 128 partitions run ~2x
                    # faster than 121 (SDMA engine load balance), so x0/W are
                    # host-padded with zero rows 121..127 and K=128 matmuls
PADH = NCORES * RPC + KH - 1  # 68
NG = 15             # groups of 4 x-positions per row
CHUNKS = ((0, 4), (4, 4), (8, 4), (12, 3))  # (first group, n groups) per PSUM chunk
FXB = RX * B        # 7680 elements per patch plane
RW = RX * O         # 1920 weight elems per output row per partition

_cache = {}


def _build():
    import concourse.bass as bass
    import concourse.bacc as bacc
    import concourse.tile as tile
    import concourse.mybir as mybir

    f32 = mybir.dt.float32
    din = mybir.dt.bfloat16
    nc = bacc.Bacc("TRN2", target_bir_lowering=False, debug=False,
                   num_devices=NCORES)
    x0_d = nc.dram_tensor("x0", [KP, 2 * FXB], din, kind="ExternalInput")
    wh_d = nc.dram_tensor("wh", [KP, RPC * RW], din, kind="ExternalInput")
    oc_d = nc.dram_tensor("oc", [4, 128, 2 * NG * B], din, kind="ExternalOutput")

    with tile.TileContext(nc) as tc:
        with (
            tc.tile_pool(name="const", bufs=1) as cpool,
            tc.tile_pool(name="ps", bufs=6, space=bass.MemorySpace.PSUM) as ppool,
        ):
            # two buffers as column halves: A = input rows 0-7 (output rows
            # 0-3), B = input rows 4-11 (output rows 4-7); + ones + 7 pad rows
            xp = cpool.tile([KP, 2 * FXB], din)
            wt = cpool.tile([KP, RPC * RW], din)  # all 8 rows' weights (padded)
            ots = [cpool.tile([128, 2 * NG * B], din, name=f"ot{m}")
                   for m in range(4)]

            # --- input loads: few, large DMAs (sem-pool sharing makes every
            # extra DMA a false-serialization hazard).  sync = x-planes
            # (+ ring advances later), scalar = host-padded W per pair
            # (+ stores later) ---
            nc.scalar.dma_start(wt[:, 0:2 * RW], wh_d[:, 0:2 * RW])
            nc.sync.dma_start(xp[:, 0:FXB], x0_d[:, 0:FXB])        # buffer A
            nc.scalar.dma_start(wt[:, 2 * RW:4 * RW], wh_d[:, 2 * RW:4 * RW])
            nc.sync.dma_start(xp[:, FXB:2 * FXB], x0_d[:, FXB:2 * FXB])  # B
            nc.scalar.dma_start(wt[:, 4 * RW:6 * RW], wh_d[:, 4 * RW:6 * RW])
            nc.scalar.dma_start(wt[:, 6 * RW:8 * RW], wh_d[:, 6 * RW:8 * RW])

            for kk in range(RPC):
                m, second = kk // 2, kk % 2
                ot = ots[m]
                xoff = 0 if kk < 4 else FXB  # buffer A or B
                for ci, (g0, gn) in enumerate(CHUNKS):
                    pt = ppool.tile([128, 4 * B], f32)
                    for gs in range(gn):
                        for j in range(4):
                            xpos = (g0 + gs) * 4 + j
                            nc.tensor.matmul(
                                pt[32 * j:32 * (j + 1), gs * B:(gs + 1) * B],
                                wt[:, (kk * RX + xpos) * O:(kk * RX + xpos + 1) * O],
                                xp[:, xoff + xpos * B:xoff + (xpos + 1) * B],
                                tile_position=(0, 32 * j),
                            )
                    dst = ot[:, second * NG * B + g0 * B:
                             second * NG * B + (g0 + gn) * B]
                    if ci % 2:
                        nc.scalar.copy(dst, pt[:, :gn * B])
                    else:
                        nc.vector.tensor_copy(dst, pt[:, :gn * B])
                if kk >= 6:
                    # split the final pair's store per row to shorten the tail
                    nc.sync.dma_start(
                        oc_d[m][:, second * NG * B:(second + 1) * NG * B],
                        ot[:, second * NG * B:(second + 1) * NG * B])
                elif second:
                    nc.sync.dma_start(oc_d[m], ot[:])

    nc.compile()
    return nc


def _get_nc():
    if "nc" not in _cache:
        _cache["nc"] = _build()
    return _cache["nc"]


def _prep_inputs(x, W, b):
    import ml_dtypes
    bf = ml_dtypes.bfloat16
    x = np.asarray(x, np.float32)
    W = np.asarray(W, np.float32)
    b = np.asarray(b, np.float32)
    xh = np.zeros((PADH, C, WIDTH, B), np.float32)
    xh[:H] = x.transpose(2, 1, 3, 0)  # [row, c, w, batch]
    # patch planes: xpr_full[r, c*KW+dx, x*B+b] = xh[r, c, x+dx, b]
    xpr_full = np.zeros((PADH, C, KW, RX, B), np.float32)
    for dx in range(KW):
        xpr_full[:, :, dx] = xh[:, :, dx:dx + RX]
    xpr_full = xpr_full.reshape(PADH, NPL, FXB).astype(bf)
    Wfull = W.transpose(0, 3, 1, 2)  # [RY, 75, RX, O]
    ones_row = np.ones((1, FXB), bf)
    in_maps = []
    for i in range(NCORES):
        # host-padded W tile image: [KP, RPC, RX, O]; output row k reads
        # buffer A (k<4, input row r at partition 15r) or buffer B (k>=4,
        # input row r at partition 15(r-4)), so its 75 live rows sit at
        # partitions [15(k%4), 15(k%4)+75) - never wrapping; bias at 120,
        # rows 121..127 zero (128-partition DMA alignment)
        whc = np.zeros((KP, RPC, RX, O), np.float32)
        for k in range(RPC):
            y = RPC * i + k
            if y < RY:
                w5 = Wfull[y].reshape(C, KH, KW, RX, O)  # (c, dy, dx, x, o)
                flat = w5.transpose(1, 0, 2, 3, 4).reshape(75, RX, O)
                p0 = NPL * (k % 4)
                whc[p0:p0 + 75, k] = flat
                whc[KC - 1, k] = b[y]
        pad = np.zeros((KP - KC, FXB), bf)
        xa = np.concatenate(
            [xpr_full[RPC * i:RPC * i + NSLOT].reshape(NSLOT * NPL, FXB),
             ones_row, pad], axis=0)
        xb = np.concatenate(
            [xpr_full[RPC * i + 4:RPC * i + 12].reshape(NSLOT * NPL, FXB),
             ones_row, pad], axis=0)
        in_maps.append({
            "x0": np.ascontiguousarray(
                np.concatenate([xa[:, None, :], xb[:, None, :]], axis=1)
                .reshape(KP, 2 * FXB)),
            "wh": np.ascontiguousarray(
                whc.reshape(KP, RPC * RW)).astype(bf),
        })
    return in_maps


def kernel(x, W, b):
    from concourse.bass_utils import run_bass_kernel_spmd

    nc = _get_nc()
    in_maps = _prep_inputs(x, W, b)
    br = run_bass_kernel_spmd(nc, in_maps, list(range(NCORES)),
                              **_cache.get("run_kwargs", {}))
    _cache["last_run"] = br
    oc = np.stack([np.asarray(br.results[i]["oc"]) for i in range(NCORES)])
    # oc: [i, m, p=32j+o, k2*NG*B + g*B + b] -> out[b, o, y=8i+2m+k2, x=4g+j]
    oc = oc.reshape(NCORES, 4, 4, O, 2, NG, B).astype(np.float32)
    out = oc.transpose(6, 3, 0, 1, 4, 5, 2).reshape(B, O, NCORES * RPC, RX)
    return np.ascontiguousarray(out[:, :, :RY, :])


# revision 27
# speedup vs baseline: 1.0939x; 1.0939x over previous
"""Locally-connected 2D layer on 8 Trainium2 NeuronCores.

Problem: x[128,3,64,64] f32, per-position weights W[60,60,32,75], bias b[60,60,32]
  out[b,o,y,x] = sum_k patches[b,y,x,k] * W[y,x,o,k] + b[y,x,o],  k=(c,dy,dx)

Strategy (spatial sharding over output rows, 8 rows/core, memory-regime):
  - mod-8 ring of input-row patch planes on SBUF partitions 0..119 (8 slots x
    15 (c,dx)-planes), ones row at partition 120 -> contraction K=121.  Input
    row r lives in slot r%8; output row k multiplies slots k..k+4 (mod 8) with
    nonzero weights and the other 45 plane rows with zeros, so the rhs window
    is always the full fixed [0,121) partition range - no ring rotation, no
    wraparound, no SBUF->SBUF copies.
  - W is stored per-row UNPADDED in HBM ([75,1920] per output row) and DMA'd
    into a zero-memset [121, 8*1920] tile at the row's (possibly wrapped)
    partition stripes; bias is one [1, 8*1920] row at partition 120 (ones).
  - Ring advances (input rows 8..11 -> slots 0..3) are plain HBM loads with
    ~3 output rows of scheduling slack, column-halved and issued right after
    the last reader's matmuls.
  - All DMAs are large-ish and spread over the sync/scalar/gpsimd queues for
    parallel dispatch + deeper SDMA pipelining (~250 GB/s aggregate).
  - Per output row: 15 groups of 4 column-tiled matmuls (lhsT=W[121,32],
    rhs=XP[121,128] -> out[32o,128b] at PSUM partitions 32j); PSUM->SBUF
    copies (f32->bf16) rotate over vector/scalar/gpsimd; bf16 stores per pair.
"""

import numpy as np

B, C, H, WIDTH = 128, 3, 64, 64
KH = KW = 5
RY = RX = 60
O = 32
NCORES = 8
RPC = 8             # output rows computed per core (8*8=64, last 4 dropped)
NSLOT = 8           # ring slots; contraction = 8*15 + 1(ones) = 121
NPL = KW * C        # 15 planes per input row
KC = NSLOT * NPL + 1  # 121 live contraction rows (ones at 120)
KP = 128            # tile partition count; DMAs at 128 partitions run ~2x
                    # faster than 121 (SDMA engine load balance), so x0/W are
                    # host-padded with zero rows 121..127 and K=128 matmuls
PADH = NCORES * RPC + KH - 1  # 68
NG = 15             # groups of 4 x-positions per row
CHUNKS = ((0, 4), (4, 4), (8, 4), (12, 3))  # (first group, n groups) per PSUM chunk
FXB = RX * B        # 7680 elements per patch plane
RW = RX * O         # 1920 weight elems per output row per partition

_cache = {}


def _build():
    import concourse.bass as bass
    import concourse.bacc as bacc
    import concourse.tile as tile
    import concourse.mybir as mybir

    f32 = mybir.dt.float32
    din = mybir.dt.bfloat16
    nc = bacc.Bacc("TRN2", target_bir_lowering=False, debug=False,
                   num_devices=NCORES)
    x0_d = nc.dram_tensor("x0", [KP, 2 * FXB], din, kind="ExternalInput")
    wh_d = nc.dram_tensor("wh", [KP, RPC * RW], din, kind="ExternalInput")
    oc_d = nc.dram_tensor("oc", [4, 128, 2 * NG * B], din, kind="ExternalOutput")

    with tile.TileContext(nc) as tc:
        with (
            tc.tile_pool(name="const", bufs=1) as cpool,
            tc.tile_pool(name="ps", bufs=6, space=bass.MemorySpace.PSUM) as ppool,
        ):
            # two buffers as column halves: A = input rows 0-7 (output rows
            # 0-3), B = input rows 4-11 (output rows 4-7); + ones + 7 pad rows
            xp = cpool.tile([KP, 2 * FXB], din)
            wt = cpool.tile([KP, RPC * RW], din)  # all 8 rows' weights (padded)
            ots = [cpool.tile([128, 2 * NG * B], din, name=f"ot{m}")
                   for m in range(4)]

            # --- input loads: few, large DMAs (sem-pool sharing makes every
            # extra DMA a false-serialization hazard).  sync = x-planes
            # (+ ring advances later), scalar = host-padded W per pair
            # (+ stores later) ---
            # W splits taper 4/2/1/1: big descriptors early (packet-RR
            # bandwidth share tracks descriptor size), and the last-arriving
            # W bytes gate only the final row's compute
            nc.scalar.dma_start(wt[:, 0:4 * RW], wh_d[:, 0:4 * RW])
            nc.sync.dma_start(xp[:, 0:FXB], x0_d[:, 0:FXB])        # buffer A
            nc.sync.dma_start(xp[:, FXB:2 * FXB], x0_d[:, FXB:2 * FXB])  # B
            nc.scalar.dma_start(wt[:, 4 * RW:6 * RW], wh_d[:, 4 * RW:6 * RW])
            nc.scalar.dma_start(wt[:, 6 * RW:7 * RW], wh_d[:, 6 * RW:7 * RW])
            nc.scalar.dma_start(wt[:, 7 * RW:8 * RW], wh_d[:, 7 * RW:8 * RW])

            for kk in range(RPC):
                m, second = kk // 2, kk % 2
                ot = ots[m]
                xoff = 0 if kk < 4 else FXB  # buffer A or B
                for ci, (g0, gn) in enumerate(CHUNKS):
                    pt = ppool.tile([128, 4 * B], f32)
                    for gs in range(gn):
                        for j in range(4):
                            xpos = (g0 + gs) * 4 + j
                            nc.tensor.matmul(
                                pt[32 * j:32 * (j + 1), gs * B:(gs + 1) * B],
                                wt[:, (kk * RX + xpos) * O:(kk * RX + xpos + 1) * O],
                                xp[:, xoff + xpos * B:xoff + (xpos + 1) * B],
                                tile_position=(0, 32 * j),
                            )
                    dst = ot[:, second * NG * B + g0 * B:
                             second * NG * B + (g0 + gn) * B]
                    if ci % 2:
                        nc.scalar.copy(dst, pt[:, :gn * B])
                    else:
                        nc.vector.tensor_copy(dst, pt[:, :gn * B])
                if kk >= 6:
                    # final pair: store each row in two halves right behind
                    # its chunk copies to shorten the tail
                    h0 = second * NG * B
                    nc.scalar.dma_start(
                        oc_d[m][:, h0:h0 + 8 * B], ot[:, h0:h0 + 8 * B])
                    nc.scalar.dma_start(
                        oc_d[m][:, h0 + 8 * B:h0 + NG * B],
                        ot[:, h0 + 8 * B:h0 + NG * B])
                elif second:
                    nc.scalar.dma_start(oc_d[m], ot[:])

    nc.compile()
    return nc


def _get_nc():
    if "nc" not in _cache:
        _cache["nc"] = _build()
    return _cache["nc"]


def _prep_inputs(x, W, b):
    import ml_dtypes
    bf = ml_dtypes.bfloat16
    x = np.asarray(x, np.float32)
    W = np.asarray(W, np.float32)
    b = np.asarray(b, np.float32)
    xh = np.zeros((PADH, C, WIDTH, B), np.float32)
    xh[:H] = x.transpose(2, 1, 3, 0)  # [row, c, w, batch]
    # patch planes: xpr_full[r, c*KW+dx, x*B+b] = xh[r, c, x+dx, b]
    xpr_full = np.zeros((PADH, C, KW, RX, B), np.float32)
    for dx in range(KW):
        xpr_full[:, :, dx] = xh[:, :, dx:dx + RX]
    xpr_full = xpr_full.reshape(PADH, NPL, FXB).astype(bf)
    Wfull = W.transpose(0, 3, 1, 2)  # [RY, 75, RX, O]
    ones_row = np.ones((1, FXB), bf)
    in_maps = []
    for i in range(NCORES):
        # host-padded W tile image: [KP, RPC, RX, O]; output row k reads
        # buffer A (k<4, input row r at partition 15r) or buffer B (k>=4,
        # input row r at partition 15(r-4)), so its 75 live rows sit at
        # partitions [15(k%4), 15(k%4)+75) - never wrapping; bias at 120,
        # rows 121..127 zero (128-partition DMA alignment)
        whc = np.zeros((KP, RPC, RX, O), np.float32)
        for k in range(RPC):
            y = RPC * i + k
            if y < RY:
                w5 = Wfull[y].reshape(C, KH, KW, RX, O)  # (c, dy, dx, x, o)
                flat = w5.transpose(1, 0, 2, 3, 4).reshape(75, RX, O)
                p0 = NPL * (k % 4)
                whc[p0:p0 + 75, k] = flat
                whc[KC - 1, k] = b[y]
        pad = np.zeros((KP - KC, FXB), bf)
        xa = np.concatenate(
            [xpr_full[RPC * i:RPC * i + NSLOT].reshape(NSLOT * NPL, FXB),
             ones_row, pad], axis=0)
        xb = np.concatenate(
            [xpr_full[RPC * i + 4:RPC * i + 12].reshape(NSLOT * NPL, FXB),
             ones_row, pad], axis=0)
        in_maps.append({
            "x0": np.ascontiguousarray(
                np.concatenate([xa[:, None, :], xb[:, None, :]], axis=1)
                .reshape(KP, 2 * FXB)),
            "wh": np.ascontiguousarray(
                whc.reshape(KP, RPC * RW)).astype(bf),
        })
    return in_maps


def kernel(x, W, b):
    from concourse.bass_utils import run_bass_kernel_spmd

    nc = _get_nc()
    in_maps = _prep_inputs(x, W, b)
    br = run_bass_kernel_spmd(nc, in_maps, list(range(NCORES)),
                              **_cache.get("run_kwargs", {}))
    _cache["last_run"] = br
    oc = np.stack([np.asarray(br.results[i]["oc"]) for i in range(NCORES)])
    # oc: [i, m, p=32j+o, k2*NG*B + g*B + b] -> out[b, o, y=8i+2m+k2, x=4g+j]
    oc = oc.reshape(NCORES, 4, 4, O, 2, NG, B).astype(np.float32)
    out = oc.transpose(6, 3, 0, 1, 4, 5, 2).reshape(B, O, NCORES * RPC, RX)
    return np.ascontiguousarray(out[:, :, :RY, :])
